# revision 1
# baseline (speedup 1.0000x reference)
"""HGNNConv on 8 Trainium2 NeuronCores.

out = relu(D_v^-1/2 H D_e^-1 H^T D_v^-1/2 (X @ theta_w + theta_b))

Sharding: vertices (rows of X / output) split contiguously across 8 cores.
Per core: GEMM on its X slice (inv_sqrt_dv folded in), then phase 1
(vertex->edge segment sum, entries sorted by edge, one-hot matmul windows
over 128-edge blocks, per-entry rows fetched by indirect DMA gather from
the core's Xs), AllReduce of edge partials, then phase 2 (edge->vertex
segment sum, entries sorted by vertex, gather from the reduced Ye).
All diagonal scalings fold into the GEMM output / PSUM-copyback / final
relu-activation scale, so the one-hot matrices are pure 0/1.

Gather tile counts are per-block (max over the 8 cores, so the SPMD
program stays core-uniform) rather than a global max — trims padded
indirect-DMA calls, which are the bottleneck (~1us each on the GPSIMD
descriptor-generation queue).
"""
import sys

if "/opt/trn_rl_repo" not in sys.path:
    sys.path.insert(0, "/opt/trn_rl_repo")

from contextlib import ExitStack
from dataclasses import dataclass

import numpy as np

import concourse.bass as bass
import concourse.tile as tile
from concourse import bacc, mybir
from concourse.bass_utils import run_bass_kernel_spmd
from concourse.masks import make_identity

P = 128
CORES = 8


@dataclass(frozen=True)
class Cfg:
    n: int            # vertices
    m: int            # hyperedges
    d: int            # feature dim (=128)
    cores: int
    nt1: tuple        # gather tiles per edge-block (len ge)
    nt2: tuple        # gather tiles per vertex-block (len gv)

    @property
    def nv(self):
        return self.n // self.cores

    @property
    def nvp(self):
        return ((self.nv + P - 1) // P) * P

    @property
    def mp(self):
        return ((self.m + P - 1) // P) * P

    @property
    def gv(self):
        return self.nvp // P

    @property
    def ge(self):
        return self.mp // P

    @property
    def nc1(self):
        return int(sum(self.nt1))

    @property
    def nc2(self):
        return int(sum(self.nt2))


def build_kernel(cfg: Cfg):
    nc = bacc.Bacc("TRN2", target_bir_lowering=False, debug=False,
                   num_devices=cfg.cores)
    f32, i32 = mybir.dt.float32, mybir.dt.int32
    ge, gv, d = cfg.ge, cfg.gv, cfg.d
    nc1, nc2 = cfg.nc1, cfg.nc2
    off1 = np.concatenate([[0], np.cumsum(cfg.nt1)]).astype(int)
    off2 = np.concatenate([[0], np.cumsum(cfg.nt2)]).astype(int)

    xk = nc.dram_tensor("xk", [cfg.nvp, d], f32, kind="ExternalInput")
    theta = nc.dram_tensor("theta", [d, d], f32, kind="ExternalInput")
    brow = nc.dram_tensor("brow", [P, d], f32, kind="ExternalInput")
    iota = nc.dram_tensor("iota", [P, P], f32, kind="ExternalInput")
    isdw = nc.dram_tensor("isdw", [P, gv], f32, kind="ExternalInput")
    idew = nc.dram_tensor("idew", [P, ge], f32, kind="ExternalInput")
    g1i = nc.dram_tensor("g1i", [P, nc1], i32, kind="ExternalInput")
    g1s = nc.dram_tensor("g1s", [P, nc1], f32, kind="ExternalInput")
    g2i = nc.dram_tensor("g2i", [P, nc2], i32, kind="ExternalInput")
    g2s = nc.dram_tensor("g2s", [P, nc2], f32, kind="ExternalInput")
    out = nc.dram_tensor("out", [cfg.nvp, d], f32, kind="ExternalOutput")

    xs = nc.dram_tensor("xs", [cfg.nvp, d], f32)
    yep = nc.dram_tensor("yep", [cfg.mp, d], f32)
    yef = nc.dram_tensor("yef", [cfg.mp, d], f32)

    with tile.TileContext(nc) as tc, ExitStack() as ctx:
        cst = ctx.enter_context(tc.tile_pool(name="cst", bufs=1))
        sb = ctx.enter_context(tc.tile_pool(name="sb", bufs=6))
        ps = ctx.enter_context(tc.tile_pool(name="ps", bufs=2, space="PSUM"))

        ident = cst.tile([P, P], f32)
        make_identity(nc, ident[:])
        theta_t = cst.tile([P, d], f32)
        nc.sync.dma_start(theta_t[:], theta[:, :])
        brow_t = cst.tile([P, d], f32)
        nc.sync.dma_start(brow_t[:], brow[:, :])
        iota_t = cst.tile([P, P], f32)
        nc.sync.dma_start(iota_t[:], iota[:, :])
        isdw_t = cst.tile([P, gv], f32)
        nc.sync.dma_start(isdw_t[:], isdw[:, :])
        idew_t = cst.tile([P, ge], f32)
        nc.sync.dma_start(idew_t[:], idew[:, :])
        g1i_t = cst.tile([P, nc1], i32)
        nc.sync.dma_start(g1i_t[:], g1i[:, :])
        g1s_t = cst.tile([P, nc1], f32)
        nc.sync.dma_start(g1s_t[:], g1s[:, :])
        g2i_t = cst.tile([P, nc2], i32)
        nc.sync.dma_start(g2i_t[:], g2i[:, :])
        g2s_t = cst.tile([P, nc2], f32)
        nc.sync.dma_start(g2s_t[:], g2s[:, :])

        # Stage A: Xs = (X @ theta + b) * inv_sqrt_dv, written to DRAM
        ntile = cfg.nvp // P
        xg = None
        for t in range(ntile):
            if t % 4 == 0:
                nb = min(4, ntile - t)
                xg = sb.tile([P, 4, d], f32, tag="x4")
                nc.sync.dma_start(
                    xg[:, :nb, :],
                    xk[t * P:(t + nb) * P, :].rearrange("(a p) d -> p a d", p=P))
            x_t = xg[:, t % 4, :]
            xt_ps = ps.tile([P, P], f32, space="PSUM", tag="tp")
            nc.tensor.transpose(xt_ps[:], x_t[:], ident[:])
            xt_s = sb.tile([P, P], f32, tag="xt")
            nc.vector.tensor_copy(xt_s[:], xt_ps[:])
            mm = ps.tile([P, d], f32, space="PSUM", tag="mm")
            nc.tensor.matmul(mm[:], lhsT=xt_s[:], rhs=theta_t[:],
                             start=True, stop=True)
            xb = sb.tile([P, d], f32, tag="xb")
            nc.vector.tensor_tensor(
                xb[:], mm[:], brow_t[:], mybir.AluOpType.add)
            xs_t = sb.tile([P, d], f32, tag="xs")
            nc.vector.tensor_scalar(xs_t[:], xb[:], isdw_t[:, t:t + 1], None,
                                    mybir.AluOpType.mult)
            nc.sync.dma_start(xs[t * P:(t + 1) * P, :], xs_t[:])

        # Stage B: phase 1 -> yep[e,:] = inv_de[e] * sum_{i:e_i=e} Xs[v_i,:]
        for g in range(ge):
            tb = int(cfg.nt1[g])
            if tb == 0:
                continue
            acc = ps.tile([P, d], f32, space="PSUM", tag="acc1")
            for t in range(tb):
                col = int(off1[g]) + t
                gt = sb.tile([P, d], f32, tag="g1")
                nc.gpsimd.indirect_dma_start(
                    out=gt[:], out_offset=None, in_=xs[:, :],
                    in_offset=bass.IndirectOffsetOnAxis(
                        ap=g1i_t[:, col:col + 1], axis=0))
                a_t = sb.tile([P, P], f32, tag="a1")
                nc.vector.tensor_scalar(a_t[:], iota_t[:],
                                        g1s_t[:, col:col + 1], None,
                                        mybir.AluOpType.is_equal)
                nc.tensor.matmul(acc[:], lhsT=a_t[:], rhs=gt[:],
                                 start=(t == 0), stop=(t == tb - 1))
            ye_t = sb.tile([P, d], f32, tag="ye")
            nc.vector.tensor_scalar(ye_t[:], acc[:], idew_t[:, g:g + 1], None,
                                    mybir.AluOpType.mult)
            nc.sync.dma_start(yep[g * P:(g + 1) * P, :], ye_t[:])

        # Stage C: AllReduce edge partials
        nc.gpsimd.collective_compute(
            "AllReduce", mybir.AluOpType.add,
            replica_groups=[list(range(cfg.cores))],
            ins=[yep[:, :]], outs=[yef[:, :]])

        # Stage D: phase 2 -> out[v,:] = relu(isd[v] * sum_{i:v_i=v} yef[e_i,:])
        for j in range(gv):
            tb = int(cfg.nt2[j])
            o_t = sb.tile([P, d], f32, tag="o")
            if tb == 0:
                nc.vector.memset(o_t[:], 0.0)
            else:
                acc = ps.tile([P, d], f32, space="PSUM", tag="acc2")
                for t in range(tb):
                    col = int(off2[j]) + t
                    gt = sb.tile([P, d], f32, tag="g2")
                    nc.gpsimd.indirect_dma_start(
                        out=gt[:], out_offset=None, in_=yef[:, :],
                        in_offset=bass.IndirectOffsetOnAxis(
                            ap=g2i_t[:, col:col + 1], axis=0))
                    a_t = sb.tile([P, P], f32, tag="a2")
                    nc.vector.tensor_scalar(a_t[:], iota_t[:],
                                            g2s_t[:, col:col + 1], None,
                                            mybir.AluOpType.is_equal)
                    nc.tensor.matmul(acc[:], lhsT=a_t[:], rhs=gt[:],
                                     start=(t == 0), stop=(t == tb - 1))
                nc.scalar.activation(o_t[:], acc[:],
                                     mybir.ActivationFunctionType.Relu,
                                     scale=isdw_t[:, j:j + 1])
            nc.sync.dma_start(out[j * P:(j + 1) * P, :], o_t[:])

    nc.compile()
    return nc


def _group_pad(core, blk, pos_src, slot_src, n_cores, n_blk, nt):
    """Scatter per-(core,block) entry lists into [C, P, sum(nt)] layouts with
    per-block tile capacities nt[g]*P; column of (block g, tile t) = off[g]+t."""
    off = np.concatenate([[0], np.cumsum(nt)]).astype(np.int64)
    ncols = int(off[-1])
    idx_arr = np.zeros((n_cores, ncols, P), np.int32)
    slot_arr = np.full((n_cores, ncols, P), -1.0, np.float32)
    key = core * n_blk + blk
    cnt = np.bincount(key, minlength=n_cores * n_blk)
    start = np.zeros(n_cores * n_blk + 1, np.int64)
    start[1:] = np.cumsum(cnt)
    pos = np.arange(len(core)) - start[key]
    col = off[blk] + pos // P
    lane = pos % P
    idx_arr[core, col, lane] = pos_src
    slot_arr[core, col, lane] = slot_src
    idx_w = idx_arr.transpose(0, 2, 1).copy()    # [C, P, ncols]
    slot_w = slot_arr.transpose(0, 2, 1).copy()
    return idx_w, slot_w


def prepare(X, theta_w, theta_b, v_idx, e_idx, n, m, d, n_cores):
    v = np.asarray(v_idx, np.int64)
    e = np.asarray(e_idx, np.int64)
    X = np.asarray(X, np.float32)

    d_v = np.bincount(v, minlength=n).astype(np.float32)
    d_e = np.bincount(e, minlength=m).astype(np.float32)
    with np.errstate(divide="ignore"):
        isd = np.where(d_v > 0, d_v ** -0.5, 0.0).astype(np.float32)
        ide = np.where(d_e > 0, 1.0 / d_e, 0.0).astype(np.float32)

    nv = n // n_cores
    core = (v // nv).astype(np.int64)

    # phase 1: per-core entries sorted by e, grouped by 128-edge block
    o1 = np.lexsort((e, core))
    c1, e1, v1 = core[o1], e[o1], v[o1]
    eb = e1 // P
    mp = ((m + P - 1) // P) * P
    ge = mp // P
    cnt1 = np.bincount(c1 * ge + eb, minlength=n_cores * ge).reshape(n_cores, ge)
    nt1 = tuple(int(x) for x in np.ceil(cnt1.max(axis=0) / P).astype(int))
    g1i, g1s = _group_pad(c1, eb, (v1 - c1 * nv).astype(np.int32),
                          (e1 - eb * P).astype(np.float32), n_cores, ge, nt1)

    # phase 2: per-core entries sorted by v, grouped by 128-vertex block
    o2 = np.argsort(v, kind="stable")
    c2, e2, v2 = core[o2], e[o2], v[o2]
    lv = v2 - c2 * nv
    jb = lv // P
    nvp = ((nv + P - 1) // P) * P
    gv = nvp // P
    cnt2 = np.bincount(c2 * gv + jb, minlength=n_cores * gv).reshape(n_cores, gv)
    nt2 = tuple(int(x) for x in np.ceil(cnt2.max(axis=0) / P).astype(int))
    g2i, g2s = _group_pad(c2, jb, e2.astype(np.int32),
                          (lv - jb * P).astype(np.float32), n_cores, gv, nt2)

    cfg = Cfg(n=n, m=m, d=d, cores=n_cores, nt1=nt1, nt2=nt2)

    iota = np.tile(np.arange(P, dtype=np.float32), (P, 1))
    theta = np.asarray(theta_w, np.float32)
    brow = np.tile(np.asarray(theta_b, np.float32).reshape(1, d), (P, 1))
    ide_pad = np.zeros(mp, np.float32)
    ide_pad[:m] = ide
    idew = ide_pad.reshape(ge, P).T.copy()

    in_maps = []
    for k in range(n_cores):
        xk = np.zeros((nvp, d), np.float32)
        xk[:nv] = X[k * nv:(k + 1) * nv]
        isd_pad = np.zeros(nvp, np.float32)
        isd_pad[:nv] = isd[k * nv:(k + 1) * nv]
        isdw = isd_pad.reshape(gv, P).T.copy()
        in_maps.append(dict(
            xk=xk, theta=theta, brow=brow, iota=iota, isdw=isdw, idew=idew,
            g1i=np.ascontiguousarray(g1i[k]), g1s=np.ascontiguousarray(g1s[k]),
            g2i=np.ascontiguousarray(g2i[k]), g2s=np.ascontiguousarray(g2s[k]),
        ))
    return cfg, in_maps


_CACHE = {}


def kernel(X, theta_w, theta_b, v_idx, e_idx):
    N, M, D = 100000, 20000, 128
    cfg, in_maps = prepare(X, theta_w, theta_b, v_idx, e_idx, N, M, D, CORES)
    key = (cfg.nt1, cfg.nt2)
    if key not in _CACHE:
        _CACHE[key] = build_kernel(cfg)
    nc = _CACHE[key]
    res = run_bass_kernel_spmd(nc, in_maps, list(range(CORES)))
    nv = cfg.nv
    outp = np.concatenate([res.results[k]["out"][:nv] for k in range(CORES)], axis=0)
    return outp.astype(np.float32)



# revision 3
# speedup vs baseline: 1.0973x; 1.0973x over previous
"""HGNNConv on 8 Trainium2 NeuronCores — v10 (no stage A).

out = relu(D_v^-1/2 H D_e^-1 H^T D_v^-1/2 (X @ theta_w + theta_b))

Reformulation: phase 1 gathers RAW X rows (bf16 input, no pre-GEMM pass,
so the Pool/SWDGE gather stream starts immediately), folds isd*ide into
the one-hot weights (fused is_equal+mult tensor_scalar), contracts
entries with lhsT = gathered tile giving acc1T[fin, eslot], applies
theta once per edge block (matmul lhsT=theta), and keeps Ye transposed
(feature-major) through the chunked ReduceScatter+AllGather. Each chunk
is then transposed back (PE transpose) to edge-major for phase-2 row
gathers. The theta_b bias reduces to a host-computed rank-1 term
tv_v * b added via a 1-partition matmul per vertex block in phase 2.

Per-entry data movement rides dma_gather (1024 rows/call — the SWDGE
descriptor-ring limit) in bf16; collectives are chunked (3) so they hide
under the Pool-bound gather stream.
"""
import sys

if "/opt/trn_rl_repo" not in sys.path:
    sys.path.insert(0, "/opt/trn_rl_repo")

from contextlib import ExitStack
from dataclasses import dataclass

import numpy as np
import ml_dtypes

import concourse.bass as bass
import concourse.tile as tile
from concourse import bacc, mybir
from concourse.bass_utils import run_bass_kernel_spmd
from concourse.masks import make_identity

P = 128
CORES = 8
BF16 = ml_dtypes.bfloat16
NCHUNK = 2


@dataclass(frozen=True)
class Cfg:
    n: int
    m: int
    d: int
    cores: int
    nt1: tuple        # tiles per edge block (len ge)
    nt2: tuple        # nt2[c] = per-chunk tuple of tiles per vertex block
    cbl: tuple        # chunk boundaries in edge blocks

    @property
    def nv(self):
        return self.n // self.cores

    @property
    def nvp(self):
        return ((self.nv + P - 1) // P) * P

    @property
    def mp(self):
        return ((self.m + P - 1) // P) * P

    @property
    def gv(self):
        return self.nvp // P

    @property
    def ge(self):
        return self.mp // P

    @property
    def nc1(self):
        return int(sum(self.nt1))

    @property
    def nc2(self):
        return tuple(int(sum(t)) for t in self.nt2)


def build_kernel(cfg: Cfg):
    nc = bacc.Bacc("TRN2", target_bir_lowering=False, debug=False,
                   num_devices=cfg.cores)
    f32, bf16, i16 = mybir.dt.float32, mybir.dt.bfloat16, mybir.dt.int16
    ge, gv, d = cfg.ge, cfg.gv, cfg.d
    nc1 = cfg.nc1
    nc2t = int(sum(cfg.nc2))
    off1 = np.concatenate([[0], np.cumsum(cfg.nt1)]).astype(int)

    xk = nc.dram_tensor("xk", [cfg.nvp, d], bf16, kind="ExternalInput")
    theta = nc.dram_tensor("theta", [d, d], bf16, kind="ExternalInput")
    browc = nc.dram_tensor("browc", [1, d], bf16, kind="ExternalInput")
    tvrow = nc.dram_tensor("tvrow", [1, cfg.nvp], bf16, kind="ExternalInput")
    iota = nc.dram_tensor("iota", [P, P], bf16, kind="ExternalInput")
    isdw = nc.dram_tensor("isdw", [P, gv], f32, kind="ExternalInput")
    g1x = nc.dram_tensor("g1x", [P, nc1 * 8], i16, kind="ExternalInput")
    g1s = nc.dram_tensor("g1s", [P, nc1], f32, kind="ExternalInput")
    g1w = nc.dram_tensor("g1w", [P, nc1], f32, kind="ExternalInput")
    g2x = nc.dram_tensor("g2x", [P, nc2t * 8], i16, kind="ExternalInput")
    g2s = nc.dram_tensor("g2s", [P, nc2t], f32, kind="ExternalInput")
    out = nc.dram_tensor("out", [cfg.nvp, d], f32, kind="ExternalOutput")

    crows = [(cfg.cbl[c + 1] - cfg.cbl[c]) * P for c in range(NCHUNK)]
    yepT = [nc.dram_tensor(f"yepT{c}", [P, crows[c]], bf16)
            for c in range(NCHUNK)]
    yrsT = [nc.dram_tensor(f"yrsT{c}", [P // cfg.cores, crows[c]], bf16)
            for c in range(NCHUNK)]
    yefT = [nc.dram_tensor(f"yefT{c}", [P, crows[c]], bf16)
            for c in range(NCHUNK)]
    yef = [nc.dram_tensor(f"yef{c}", [crows[c], d], bf16)
           for c in range(NCHUNK)]

    with tile.TileContext(nc) as tc, ExitStack() as ctx:
        cst = ctx.enter_context(tc.tile_pool(name="cst", bufs=1))
        psa = ctx.enter_context(tc.tile_pool(name="psa", bufs=2, space="PSUM"))
        psy = ctx.enter_context(tc.tile_pool(name="psy", bufs=2, space="PSUM"))
        psb = ctx.enter_context(tc.tile_pool(name="psb", bufs=2, space="PSUM"))
        p1 = ctx.enter_context(tc.tile_pool(name="p1", bufs=5))
        p1c = ctx.enter_context(tc.tile_pool(name="p1c", bufs=1))
        ygp = ctx.enter_context(tc.tile_pool(name="yg", bufs=2))
        tbp = ctx.enter_context(tc.tile_pool(name="tb", bufs=2))
        p2 = ctx.enter_context(tc.tile_pool(name="p2", bufs=5))
        p2c = ctx.enter_context(tc.tile_pool(name="p2c", bufs=1))
        ogp = ctx.enter_context(tc.tile_pool(name="og", bufs=2))

        ident = cst.tile([P, P], bf16)
        make_identity(nc, ident[:])
        theta_t = cst.tile([P, d], bf16)
        nc.sync.dma_start(theta_t[:], theta[:, :])
        browc_t = cst.tile([1, d], bf16)
        nc.sync.dma_start(browc_t[:], browc[:, :])
        tvrow_t = cst.tile([1, cfg.nvp], bf16)
        nc.sync.dma_start(tvrow_t[:], tvrow[:, :])
        iota_t = cst.tile([P, P], bf16)
        nc.sync.dma_start(iota_t[:], iota[:, :])
        isdw_t = cst.tile([P, gv], f32)
        nc.sync.dma_start(isdw_t[:], isdw[:, :])

        g1x_t = p1c.tile([P, nc1 * 8], i16)
        nc.sync.dma_start(g1x_t[:, 0:64 * 8], g1x[:, 0:64 * 8])
        nc.sync.dma_start(g1x_t[:, 64 * 8:], g1x[:, 64 * 8:])
        g1s_t = p1c.tile([P, nc1], f32)
        nc.sync.dma_start(g1s_t[:], g1s[:, :])
        g1w_t = p1c.tile([P, nc1], f32)
        nc.sync.dma_start(g1w_t[:], g1w[:, :])

        def transpose_back(c):
            nblk = crows[c] // P
            h0 = (nblk + 1) // 2
            tbl_cols = h0 * P
            GB = 8
            ybg = None
            for (b_lo, b_hi) in ((0, h0), (h0, nblk)):
                nbh = b_hi - b_lo
                tbl = tbp.tile([P, tbl_cols], bf16, tag="tbl")
                nc.sync.dma_start(
                    tbl[:, :nbh * P], yefT[c][:, b_lo * P:b_hi * P])
                for i in range(nbh):
                    tp = psy.tile([P, P], bf16, space="PSUM", tag="tp")
                    nc.tensor.transpose(
                        tp[:], tbl[:, i * P:(i + 1) * P], ident[:])
                    if i % GB == 0:
                        ybg = tbp.tile([P, GB, P], bf16, tag="ybg")
                    nc.scalar.activation(
                        ybg[:, i % GB, :], tp[:],
                        mybir.ActivationFunctionType.Copy)
                    if i % GB == GB - 1 or i == nbh - 1:
                        b0 = (i // GB) * GB
                        nb2 = i - b0 + 1
                        r0 = (b_lo + b0) * P
                        nc.sync.dma_start(
                            yef[c][r0:r0 + nb2 * P, :].rearrange(
                                "(a p) d -> p a d", p=P),
                            ybg[:, :nb2, :])

        # ---- Phase 1: acc1T[fin, eslot] = sum_entries w * X[v], then theta ----
        GY = 16
        GC = 8
        col2blk = np.repeat(np.arange(ge), cfg.nt1)
        flush_after = set(cfg.cbl[1:])
        yg = None
        acc = None
        gt = None
        c0 = -1
        gb = cfg.cbl[0]
        for col in range(nc1):
            if col % GC == 0:
                c0 = col
                ntk = min(GC, nc1 - c0)
                gt = p1.tile([P, ntk, d], bf16, tag="g1")
                nc.gpsimd.dma_gather(
                    gt[:, :, :], xk[:, :],
                    g1x_t[:, c0 * 8:(c0 + ntk) * 8],
                    ntk * P, ntk * P, d)
            g = int(col2blk[col])
            first = col == int(off1[g])
            last = col == int(off1[g + 1]) - 1
            if first and g == (cfg.cbl[1] + cfg.cbl[2]) // 2:
                transpose_back(0)
            if first:
                acc = psa.tile([P, d], f32, space="PSUM", tag="acc1")
            a_t = p1.tile([P, P], bf16, tag="a1")
            nc.vector.tensor_scalar(
                a_t[:], iota_t[:], g1s_t[:, col:col + 1], g1w_t[:, col:col + 1],
                mybir.AluOpType.is_equal, mybir.AluOpType.mult)
            nc.tensor.matmul(acc[:], lhsT=gt[:, col - c0, :], rhs=a_t[:],
                             start=first, stop=last)
            if last:
                a1s = p1.tile([P, P], bf16, tag="a1s")
                nc.scalar.activation(a1s[:], acc[:],
                                     mybir.ActivationFunctionType.Copy)
                yet = psy.tile([P, P], f32, space="PSUM", tag="yet")
                nc.tensor.matmul(yet[:], lhsT=theta_t[:], rhs=a1s[:],
                                 start=True, stop=True)
                if g == gb:
                    yg = ygp.tile([P, GY, P], bf16, tag="ygt")
                nc.scalar.activation(yg[:, g - gb, :], yet[:],
                                     mybir.ActivationFunctionType.Copy)
                if g - gb == GY - 1 or g == ge - 1 or (g + 1) in flush_after:
                    nb = g - gb + 1
                    cc = int(np.searchsorted(
                        np.array(cfg.cbl[1:]), g, side="right"))
                    lb = gb - cfg.cbl[cc]
                    nc.sync.dma_start(
                        yepT[cc][:, lb * P:(lb + nb) * P].rearrange(
                            "p (a q) -> p a q", q=P),
                        yg[:, :nb, :])
                    gb = g + 1
                if (g + 1) in flush_after:
                    c = list(cfg.cbl).index(g + 1) - 1
                    nc.gpsimd.collective_compute(
                        "ReduceScatter", mybir.AluOpType.add,
                        replica_groups=[list(range(cfg.cores))],
                        ins=[yepT[c][:, :]], outs=[yrsT[c][:, :]])
                    nc.gpsimd.collective_compute(
                        "AllGather", mybir.AluOpType.bypass,
                        replica_groups=[list(range(cfg.cores))],
                        ins=[yrsT[c][:, :]], outs=[yefT[c][:, :]])
        # ---- Phase 2: K passes over vertex blocks ----
        g2s_t = p2c.tile([P, nc2t], f32)
        nc.sync.dma_start(g2s_t[:], g2s[:, :])
        g2x_t = p2c.tile([P, nc2t * 8], i16)
        nc.sync.dma_start(g2x_t[:], g2x[:, :])
        xv = p2c.tile([P, gv, d], bf16)

        transpose_back(1)

        GO = 14
        base = 0
        for c in range(NCHUNK):
            pass
            ntc = cfg.nt2[c]
            ncc = int(sum(ntc))
            off2 = base + np.concatenate([[0], np.cumsum(ntc)]).astype(int)
            col2blk2 = np.repeat(np.arange(gv), ntc)
            og = None
            acc = None
            gt = None
            c0 = -1
            for cl in range(ncc):
                col = base + cl
                if cl % GC == 0:
                    c0 = col
                    ntk = min(GC, ncc - cl)
                    gt = p2.tile([P, ntk, d], bf16, tag="g2")
                    nc.gpsimd.dma_gather(
                        gt[:, :, :], yef[c][:, :],
                        g2x_t[:, c0 * 8:(c0 + ntk) * 8],
                        ntk * P, ntk * P, d)
                j = int(col2blk2[cl])
                first = col == int(off2[j])
                last = col == int(off2[j + 1]) - 1
                if first:
                    acc = psb.tile([P, d], f32, space="PSUM", tag="acc2")
                    if c == 0:
                        # bias: acc += tv[vslot] * b[f]  (1-partition matmul)
                        nc.tensor.matmul(
                            acc[:], lhsT=tvrow_t[:, j * P:(j + 1) * P],
                            rhs=browc_t[:], start=True, stop=False)
                a_t = p2.tile([P, P], bf16, tag="a2")
                nc.vector.tensor_scalar(
                    a_t[:], iota_t[:], g2s_t[:, col:col + 1], None,
                    mybir.AluOpType.is_equal)
                nc.tensor.matmul(acc[:], lhsT=a_t[:], rhs=gt[:, col - c0, :],
                                 start=(first and c != 0), stop=last)
                if last:
                    if c == 0:
                        nc.vector.tensor_copy(xv[:, j, :], acc[:])
                    elif c < NCHUNK - 1:
                        nc.vector.tensor_tensor(
                            xv[:, j, :], acc[:], xv[:, j, :],
                            mybir.AluOpType.add)
                    else:
                        xvf = p2.tile([P, d], f32, tag="xvf")
                        nc.vector.tensor_tensor(
                            xvf[:], acc[:], xv[:, j, :], mybir.AluOpType.add)
                        if j % GO == 0:
                            og = ogp.tile([P, GO, d], f32, tag="ogt")
                        nc.scalar.activation(
                            og[:, j % GO, :], xvf[:],
                            mybir.ActivationFunctionType.Relu,
                            scale=isdw_t[:, j:j + 1])
                        if j % GO == GO - 1 or j == gv - 1:
                            b0 = (j // GO) * GO
                            nb = j - b0 + 1
                            nc.sync.dma_start(
                                out[:, :].rearrange(
                                    "(p a) d -> p a d", p=P)[:, b0:b0 + nb, :],
                                og[:, :nb, :])
            base += ncc

    nc.compile()
    return nc


def _streams(core, blk, idx_src, slot_src, n_cores, n_blk, nt, pad_idx,
             w_src=None):
    off = np.concatenate([[0], np.cumsum(nt)]).astype(np.int64)
    ncols = int(off[-1])
    idx_arr = np.full((n_cores, ncols, P), pad_idx, np.int16)
    slot_arr = np.full((n_cores, ncols, P), -1.0, np.float32)
    w_arr = np.zeros((n_cores, ncols, P), np.float32)
    key = core * n_blk + blk
    cnt = np.bincount(key, minlength=n_cores * n_blk)
    start = np.zeros(n_cores * n_blk + 1, np.int64)
    start[1:] = np.cumsum(cnt)
    pos = np.arange(len(core)) - start[key]
    col = off[blk] + pos // P
    lane = pos % P
    idx_arr[core, col, lane] = idx_src
    slot_arr[core, col, lane] = slot_src
    if w_src is not None:
        w_arr[core, col, lane] = w_src
    idx_stream = idx_arr.reshape(n_cores, ncols * P)
    slot_w = slot_arr.transpose(0, 2, 1).copy()
    w_w = w_arr.transpose(0, 2, 1).copy()
    return idx_stream, slot_w, w_w


def _wrap_idxs(stream):
    C, n = stream.shape
    a = stream.reshape(C, n // 16, 16).transpose(0, 2, 1)
    return np.tile(a, (1, 8, 1)).copy()


def prepare(X, theta_w, theta_b, v_idx, e_idx, n, m, d, n_cores):
    v = np.asarray(v_idx, np.int64)
    e = np.asarray(e_idx, np.int64)
    X = np.asarray(X, np.float32)

    d_v = np.bincount(v, minlength=n).astype(np.float32)
    d_e = np.bincount(e, minlength=m).astype(np.float32)
    with np.errstate(divide="ignore"):
        isd = np.where(d_v > 0, d_v ** -0.5, 0.0).astype(np.float32)
        ide = np.where(d_e > 0, 1.0 / d_e, 0.0).astype(np.float32)

    nv = n // n_cores
    nvp = ((nv + P - 1) // P) * P
    mp = ((m + P - 1) // P) * P
    ge, gv = mp // P, nvp // P
    core = (v // nv).astype(np.int64)

    # phase 1
    o1 = np.lexsort((e, core))
    c1, e1, v1 = core[o1], e[o1], v[o1]
    eb = e1 // P
    cnt1 = np.bincount(c1 * ge + eb, minlength=n_cores * ge).reshape(n_cores, ge)
    nt1 = tuple(int(x) for x in np.ceil(cnt1.max(axis=0) / P).astype(int))
    w1 = (isd[v1] * ide[e1]).astype(np.float32)
    s1, g1slot, g1wv = _streams(c1, eb, (v1 - c1 * nv).astype(np.int16),
                                (e1 - eb * P).astype(np.float32),
                                n_cores, ge, nt1, pad_idx=nv, w_src=w1)
    g1idx = _wrap_idxs(s1)

    cbl = (0, int(round(ge * 0.35)), ge)

    # phase 2, per chunk
    o2 = np.argsort(v, kind="stable")
    c2, e2, v2 = core[o2], e[o2], v[o2]
    lv2 = v2 - c2 * nv
    jb = lv2 // P
    ebl2 = e2 // P
    chunk_of = np.searchsorted(np.array(cbl[1:]), ebl2, side="right")
    nt2 = []
    s2_parts, slot2_parts = [], []
    for c in range(NCHUNK):
        mask = chunk_of == c
        cc, ec, jc, lc = c2[mask], e2[mask], jb[mask], lv2[mask]
        cnt2 = np.bincount(cc * gv + jc, minlength=n_cores * gv).reshape(n_cores, gv)
        ntc = tuple(int(x) for x in np.ceil(cnt2.max(axis=0) / P).astype(int))
        sC, slotC, _ = _streams(cc, jc, (ec - cbl[c] * P).astype(np.int16),
                                (lc - jc * P).astype(np.float32),
                                n_cores, gv, ntc,
                                pad_idx=(cbl[c + 1] - cbl[c]) * P - 1)
        nt2.append(ntc)
        s2_parts.append(sC)
        slot2_parts.append(slotC)
    s2 = np.concatenate(s2_parts, axis=1)
    g2slot = np.concatenate(slot2_parts, axis=2)
    g2idx = _wrap_idxs(s2)

    cfg = Cfg(n=n, m=m, d=d, cores=n_cores, nt1=nt1, nt2=tuple(nt2), cbl=cbl)

    # bias rank-1 term: tv_v = sum_{e in v} ide_e * sG_e, sG_e = sum isd_v
    sG = np.zeros(m, np.float32)
    np.add.at(sG, e, isd[v])
    tv = np.zeros(n, np.float32)
    np.add.at(tv, v, ide[e] * sG[e])

    iota = np.tile(np.arange(P, dtype=np.float32), (P, 1)).astype(BF16)
    theta = np.asarray(theta_w, np.float32).astype(BF16)
    browc = np.asarray(theta_b, np.float32).reshape(1, d).astype(BF16)

    in_maps = []
    for k in range(n_cores):
        xkv = np.zeros((nvp, d), BF16)
        xkv[:nv] = X[k * nv:(k + 1) * nv].astype(BF16)
        isd_pad = np.zeros(nvp, np.float32)
        isd_pad[:nv] = isd[k * nv:(k + 1) * nv]
        isdw = isd_pad.reshape(gv, P).T.copy()
        tv_pad = np.zeros(nvp, np.float32)
        tv_pad[:nv] = tv[k * nv:(k + 1) * nv]
        in_maps.append(dict(
            xk=xkv, theta=theta, browc=browc,
            tvrow=tv_pad.reshape(1, nvp).astype(BF16),
            iota=iota, isdw=isdw,
            g1x=np.ascontiguousarray(g1idx[k]),
            g1s=np.ascontiguousarray(g1slot[k]),
            g1w=np.ascontiguousarray(g1wv[k]),
            g2x=np.ascontiguousarray(g2idx[k]),
            g2s=np.ascontiguousarray(g2slot[k]),
        ))
    return cfg, in_maps


_CACHE = {}


def kernel(X, theta_w, theta_b, v_idx, e_idx):
    N, M, D = 100000, 20000, 128
    cfg, in_maps = prepare(X, theta_w, theta_b, v_idx, e_idx, N, M, D, CORES)
    key = (cfg.nt1, cfg.nt2, cfg.cbl)
    if key not in _CACHE:
        _CACHE[key] = build_kernel(cfg)
    nc = _CACHE[key]
    res = run_bass_kernel_spmd(nc, in_maps, list(range(CORES)))
    nv = cfg.nv
    ga = cfg.nvp // P
    outs = []
    for k in range(CORES):
        o = res.results[k]["out"].reshape(P, ga, D).transpose(1, 0, 2).reshape(
            cfg.nvp, D)
        outs.append(o[:nv])
    return np.concatenate(outs, axis=0).astype(np.float32)
